# revision 42
# baseline (speedup 1.0000x reference)
"""Trainium2 Bass kernel for nn_AttentionWithTime (differential-attention block).

Sharding: data-parallel over batch B=8 -> one batch element per NeuronCore,
no collectives.

v3: all weights are quantized to fp8(e4m3), scaled, and packed into their
on-chip DoubleRow layouts on the HOST (numpy); LN1/LNf gain+bias are folded
into Wqkv/Wf1 host-side, and bias vectors arrive pre-transposed into
per-partition columns.  The device kernel just DMAs weights straight into
their resident SBUF tiles (7.3MB fp8 instead of 29MB f32) and spends no
engine time on weight casts or bias transposes.

Attention pipeline (per head h, query tile nt):
  scores/exp/U (h, nt) | v-proj (h+1, nt) | qk-proj (h+2) | U^T built with PE
  transposes grouped per key-tile (each PSUM->SBUF copy moves [128,512])
  | attn@v (h-1) -> merge (PSUM-accumulated over heads) + LNf -> FFN.
fp8 tensors carry power-of-2 scales folded into the host-packed weights.
"""
import numpy as np

import concourse.bass as bass
import concourse.mybir as mybir
import concourse.tile as tile
from concourse import bacc
from concourse.masks import make_identity

B, N, D, H, DH, DE, DT = 8, 1024, 512, 8, 64, 2048, 256
DQKV = 6144
NT = N // 128
FT = D // 128
EPS = 1e-5
SCALE = DH ** -0.5

SX = 32.0
SQK = 32.0
SU = 4096.0
SV = 64.0
SO = 512.0
SA = 32.0
SW_QKV = 4096.0
SW_M = 8192.0
SW_F1 = 4096.0
SW_F2 = 8192.0

CQ = SQK / (SX * SW_QKV)
ESC = SCALE / (SQK * SQK)
CV = SV / (SX * SW_QKV)
CO = SO / (SU * SV)
CM = 1.0 / (SO * SW_M)
CF1 = 1.0 / (SX * SW_F1)
CF2 = 1.0 / (SA * SW_F2)

f32 = mybir.dt.float32
bf16 = mybir.dt.bfloat16
fp8 = mybir.dt.float8e4
AF = mybir.ActivationFunctionType
ALU = mybir.AluOpType
PM = mybir.MatmulPerfMode

FP8NP = mybir.dt.np(fp8)


# ---------------------------------------------------------------------------
# host-side input prep: fold LN affine params into weights, quantize to fp8,
# pack into the exact SBUF tile layouts, pre-transpose bias vectors.
# ---------------------------------------------------------------------------
def prepare_inputs(inputs):
    f = lambda k: np.asarray(inputs[k], np.float32)
    lam = float(np.asarray(inputs["lam"]))
    Wqkv, bqkv = f("Wqkv"), f("bqkv")
    g1, b1 = f("ln1_g"), f("ln1_b")
    Wq_eff = g1[:, None] * Wqkv
    bq_eff = bqkv + b1 @ Wqkv
    Wm = f("Wm")
    Wf1, bf1 = f("Wf1"), f("bf1")
    gf, bfb = f("lnf_g"), f("lnf_b")
    Wf1_eff = gf[:, None] * Wf1
    bf1_eff = bf1 + bfb @ Wf1
    Wf2 = f("Wf2")

    # qk columns packed per head: dcol = hh*128 + sub*64 <- scol = sub*512 + hh*64
    wqp = np.empty((2, 128, 2, 1024), np.float32)
    wkp = np.empty((2, 128, 2, 1024), np.float32)
    for g in range(2):
        for i in range(2):
            rows = Wq_eff[(2 * g + i) * 128:(2 * g + i + 1) * 128]
            for hh in range(8):
                for sub in range(2):
                    dcol = hh * 128 + sub * 64
                    scol = sub * 512 + hh * 64
                    wqp[g, :, i, dcol:dcol + 64] = rows[:, scol:scol + 64]
                    wkp[g, :, i, dcol:dcol + 64] = rows[:, 1024 + scol:1024 + scol + 64]
    wqp = (wqp * SW_QKV).astype(FP8NP)
    wkp = (wkp * SW_QKV).astype(FP8NP)

    wvp = np.empty((H, 2, 128, 2, 512), np.float32)
    for h in range(H):
        for g in range(2):
            for i in range(2):
                wvp[h, g, :, i, :] = Wq_eff[(2 * g + i) * 128:(2 * g + i + 1) * 128,
                                            2048 + h * 512:2048 + (h + 1) * 512]
    wvp = (wvp * SW_QKV).astype(FP8NP)

    wmp = np.empty((H, 2, 128, 2, 512), np.float32)
    for h in range(H):
        for j in range(2):
            for i in range(2):
                r = (4 * h + 2 * j + i) * 128
                wmp[h, j, :, i, :] = Wm[r:r + 128, :]
    wmp = (wmp * SW_M).astype(FP8NP)

    wf1p = np.empty((2, 128, 2, DE), np.float32)
    for g in range(2):
        for i in range(2):
            wf1p[g, :, i, :] = Wf1_eff[(2 * g + i) * 128:(2 * g + i + 1) * 128]
    wf1p = (wf1p * SW_F1).astype(FP8NP)

    wf2p = np.empty((8, 128, 2, 512), np.float32)
    for j in range(8):
        for i in range(2):
            wf2p[j, :, i, :] = Wf2[(2 * j + i) * 128:(2 * j + i + 1) * 128]
    wf2p = (wf2p * SW_F2).astype(FP8NP)

    # bias columns (pre-transposed to per-partition layout)
    bqps = np.empty((128, 16), np.float32)
    for p in range(16):
        hh, base = p % 8, (0 if p < 8 else 1024)
        bqps[0:64, p] = bq_eff[base + hh * 64:base + (hh + 1) * 64]
        bqps[64:128, p] = bq_eff[base + 512 + hh * 64:base + 512 + (hh + 1) * 64]
    bqps *= SQK
    bvvs = bq_eff[2048:6144].reshape(32, 128).T.copy() * ((1.0 - lam) * SO)
    bf1c = bf1_eff.reshape(16, 128).T.copy()
    bt1c = f("bt1").reshape(2, 128).T.copy()

    # partition-major single-DMA blobs
    wqh = np.ascontiguousarray(wqp.transpose(1, 0, 2, 3).reshape(128, 2, 2048))
    wkh = np.ascontiguousarray(wkp.transpose(1, 0, 2, 3).reshape(128, 2, 2048))
    wvh = np.ascontiguousarray(
        wvp.transpose(2, 0, 1, 3, 4).reshape(128, 16, 1024))
    wmh = np.ascontiguousarray(
        wmp.transpose(2, 0, 1, 3, 4).reshape(128, 16, 1024))
    wf1h = np.ascontiguousarray(wf1p.transpose(1, 0, 2, 3).reshape(128, 2, 2 * DE))
    wf2h = np.ascontiguousarray(wf2p.transpose(1, 0, 2, 3).reshape(128, 8, 1024))
    cvec = np.concatenate([bqps, bvvs, bf1c, bt1c], axis=1).astype(np.float32)  # [128, 66]
    rows3 = np.ascontiguousarray(
        np.concatenate([f("bm"), f("bt2"), f("bf2")])[None, :])  # [1, 1536]
    wt1h = np.ascontiguousarray(f("Wt1").reshape(2, 128, DT).transpose(1, 0, 2))
    wt2h = np.ascontiguousarray(f("Wt2").reshape(2, 128, D).transpose(1, 0, 2))
    shared = {
        "wqh": wqh, "wkh": wkh, "wvh": wvh, "wmh": wmh, "wf1h": wf1h,
        "wf2h": wf2h, "cvec": cvec, "rows3": rows3,
        "wt1h": wt1h, "wt2h": wt2h,
    }
    x = f("x")
    t = f("t")
    in_maps = []
    for b in range(B):
        m = dict(shared)
        m["x"] = np.ascontiguousarray(x[b])
        m["cvec"] = np.ascontiguousarray(np.concatenate(
            [cvec, t[b].reshape(2, 128).T], axis=1))  # [128, 68]
        in_maps.append(m)
    return lam, in_maps


def build_program(lam: float):
    nc = bacc.Bacc("TRN2", target_bir_lowering=False, debug=False, num_devices=8)

    d = {}
    d["x_d"] = nc.dram_tensor("x", [N, D], f32, kind="ExternalInput")
    d["wqh_d"] = nc.dram_tensor("wqh", [128, 2, 2048], fp8, kind="ExternalInput")
    d["wkh_d"] = nc.dram_tensor("wkh", [128, 2, 2048], fp8, kind="ExternalInput")
    d["wvh_d"] = nc.dram_tensor("wvh", [128, 16, 1024], fp8, kind="ExternalInput")
    d["wmh_d"] = nc.dram_tensor("wmh", [128, 16, 1024], fp8, kind="ExternalInput")
    d["wf1h_d"] = nc.dram_tensor("wf1h", [128, 2, 2 * DE], fp8, kind="ExternalInput")
    d["wf2h_d"] = nc.dram_tensor("wf2h", [128, 8, 1024], fp8, kind="ExternalInput")
    d["cvec_d"] = nc.dram_tensor("cvec", [128, 68], f32, kind="ExternalInput")
    d["rows3_d"] = nc.dram_tensor("rows3", [1, 1536], f32, kind="ExternalInput")
    d["wt1h_d"] = nc.dram_tensor("wt1h", [128, 2, DT], f32, kind="ExternalInput")
    d["wt2h_d"] = nc.dram_tensor("wt2h", [128, 2, D], f32, kind="ExternalInput")
    d["y_d"] = nc.dram_tensor("y", [N, D], f32, kind="ExternalOutput")

    with tile.TileContext(nc) as tc:
        _build(tc, lam, d)
    nc.compile()
    return nc


def _build(tc, lam, d):
    nc = tc.nc
    dma = nc.sync.dma_start

    from contextlib import ExitStack
    with ExitStack() as es:
        cst = es.enter_context(tc.tile_pool(name="cst", bufs=1))
        small = es.enter_context(tc.tile_pool(name="small", bufs=8))
        otp = es.enter_context(tc.tile_pool(name="otp", bufs=1))
        ps = es.enter_context(tc.tile_pool(name="ps", bufs=1, space="PSUM"))

        _scnt = [0]

        def psS():
            _scnt[0] += 1
            return ps.tile([128, N], f32, tag=f"s{_scnt[0] % 3}", bufs=1, name="psS")

        def psUT():
            return ps.tile([128, 512], f32, tag="mm", bufs=2, name="psUT")

        def psTR():
            return ps.tile([128, FT, 128], bf16, tag="mm", bufs=2, name="psTR")

        def psMM():
            return ps.tile([128, 512], f32, tag="mm", bufs=2, name="psMM")

        def rstd_ln(dst, var_ap):
            # dst = SX / sqrt(var + eps):  sqrt((var+eps)/SX^2) then recip
            nc.scalar.activation(dst, var_ap, AF.Sqrt, bias=eps_c[:],
                                 scale=1.0 / (SX * SX))
            nc.vector.reciprocal(dst, dst)

        # ---------------- constants ----------------
        ident_bf = cst.tile([128, 128], bf16)
        make_identity(nc, ident_bf[:])
        ones1 = cst.tile([1, 128], f32)
        nc.gpsimd.memset(ones1[:], 1.0)
        eps_c = cst.tile([128, 1], f32)
        nc.gpsimd.memset(eps_c[:], EPS / (SX * SX))

        # ---------------- input/weight DMAs: one blob per tensor --------------
        x_d, y_d = d["x_d"], d["y_d"]

        xall = cst.tile([128, NT, D], f32, name="xall")
        xv = x_d[:].rearrange("(a p) c -> p a c", p=128)
        for nt in range(NT):
            dma(xall[:, nt, :], xv[:, nt, :])
        xtm = [xall[:, nt, :] for nt in range(NT)]

        cv = cst.tile([128, 68], f32, name="cvec")
        dma(cv[:], d["cvec_d"][:])
        bqpS = cv[:, 0:16]
        bvvS = cv[:, 16:48]
        bf1_c = cv[:, 48:64]
        bt1_c = cv[:, 64:66]
        tT = cv[:, 66:68]

        wq_all = cst.tile([128, 2, 2048], fp8, name="wq")
        dma(wq_all[:], d["wqh_d"][:])
        wk_all = cst.tile([128, 2, 2048], fp8, name="wk")
        dma(wk_all[:], d["wkh_d"][:])
        wv_all = cst.tile([128, 16, 1024], fp8, name="wv")
        dma(wv_all[:], d["wvh_d"][:])
        wm_all = cst.tile([128, 16, 1024], fp8, name="wm")
        dma(wm_all[:], d["wmh_d"][:])
        wf1_all = cst.tile([128, 2, 2 * DE], fp8, name="wf1")
        dma(wf1_all[:], d["wf1h_d"][:])
        wf2_all = cst.tile([128, 8, 1024], fp8, name="wf2")
        dma(wf2_all[:], d["wf2h_d"][:])

        r2 = lambda ap: ap.rearrange("p (i c) -> p i c", i=2)
        wq_dr = [r2(wq_all[:, g, :]) for g in range(2)]
        wk_dr = [r2(wk_all[:, g, :]) for g in range(2)]
        wv_dr = [[r2(wv_all[:, 2 * h + g, :]) for g in range(2)] for h in range(H)]
        wm_dr = [[r2(wm_all[:, 2 * h + j, :]) for j in range(2)] for h in range(H)]
        wf1_dr = [r2(wf1_all[:, g, :]) for g in range(2)]
        wf2_dr = [r2(wf2_all[:, j, :]) for j in range(8)]

        # ---------------- LN1 (exp table: rstd via ln+exp) -------------------
        lnxT = cst.tile([128, FT, N], fp8, name="lnxT")

        def layer_norm_tile(src, dest_all, nt):
            st6 = small.tile([128, 6], f32, tag="st6")
            nc.vector.bn_stats(out=st6[:], in_=src)
            mv = small.tile([128, 2], f32, tag="mv")
            nc.vector.bn_aggr(out=mv[:], in_=st6[:])
            rstd = small.tile([128, 1], f32, tag="rstd")
            rstd_ln(rstd[:], mv[:, 1:2])
            nm = small.tile([128, 1], f32, tag="nm")
            nc.vector.tensor_scalar(nm[:], mv[:, 0:1], rstd[:], -1.0, ALU.mult, ALU.mult)
            xn = small.tile([128, D], bf16, tag="xnorm", bufs=2)
            nc.vector.tensor_scalar(xn[:], src, rstd[:], nm[:], ALU.mult, ALU.add)
            trp = psTR()
            for ft in range(FT):
                nc.tensor.transpose(trp[:, ft, :], xn[:, ft * 128:(ft + 1) * 128],
                                    ident_bf[:])
            dest = dest_all[:, :, nt * 128:(nt + 1) * 128]
            if nt % 2 == 0:
                nc.vector.tensor_copy(dest, trp[:])
            else:
                nc.scalar.activation(dest, trp[:], AF.Identity)

        for nt in range(NT):
            layer_norm_tile(xtm[nt], lnxT, nt)

        # ---------------- time MLP ------------------------------------------
        TP1 = cst.tile([128, D], f32)
        TPy = cst.tile([128, D], f32)
        with tc.tile_pool(name="trow", bufs=1) as trow:
            wt1 = trow.tile([128, 2, DT], f32, name="wt1")
            nc.scalar.dma_start(wt1[:], d["wt1h_d"][:])
            wt2 = trow.tile([128, 2, D], f32, name="wt2")
            nc.scalar.dma_start(wt2[:], d["wt2h_d"][:])
            rows3 = trow.tile([1, 3 * D], f32, name="rows3")
            nc.scalar.dma_start(rows3[:], d["rows3_d"][:])
            bm_r = rows3[0:1, 0:D]
            bt2_r = rows3[0:1, D:2 * D]
            bf2_r = rows3[0:1, 2 * D:3 * D]
            s_cols = []
            for dc in range(2):
                l1_ps = psMM()
                for ft in range(2):
                    nc.tensor.matmul(l1_ps[:, 0:1],
                                     wt1[:, ft, dc * 128:(dc + 1) * 128],
                                     tT[:, ft:ft + 1], start=(ft == 0), stop=(ft == 1))
                s_c = small.tile([128, 1], f32, tag="s_col")
                nc.scalar.activation(s_c[:], l1_ps[:, 0:1], AF.Silu, bias=bt1_c[:, dc:dc + 1])
                s_cols.append(s_c)
            tp_ps = psMM()
            for dc in range(2):
                nc.tensor.matmul(tp_ps[0:1, :], s_cols[dc][:], wt2[:, dc, :],
                                 start=(dc == 0), stop=(dc == 1))
            row1 = trow.tile([1, D], f32)
            nc.vector.tensor_add(row1[:], tp_ps[0:1, :], bt2_r)
            rowy = trow.tile([1, D], f32)
            nc.vector.tensor_sub(rowy[:], bf2_r, row1[:])
            nc.vector.tensor_add(row1[:], row1[:], bm_r)
            for row, TP in ((row1, TP1), (rowy, TPy)):
                tp_b = psMM()
                nc.tensor.matmul(tp_b[:], ones1[:], row[:], start=True, stop=True)
                nc.vector.tensor_copy(TP[:], tp_b[:])

        # ---------------- attention phase -----------------------------------
        OTp = [[otp.tile([128, 2, N], fp8, tag="ot", bufs=16, name=f"OT_{h}_{j}")
                for j in range(2)] for h in range(H)]

        with ExitStack() as esA:
            qkp = esA.enter_context(tc.tile_pool(name="qkp", bufs=1))
            ep = esA.enter_context(tc.tile_pool(name="ep", bufs=1))
            up = esA.enter_context(tc.tile_pool(name="up", bufs=1))
            utp = esA.enter_context(tc.tile_pool(name="utp", bufs=1))
            vp = esA.enter_context(tc.tile_pool(name="vp", bufs=1))
            dgp = esA.enter_context(tc.tile_pool(name="dgp", bufs=1))

            qt = [None] * H
            kt = [None] * H
            UTp = [None] * H
            Vp = [None] * H
            dcol = [None] * H
            rcol = [None] * H
            diag = [[None] * NT for _ in range(H)]
            diag2 = [[None] * NT for _ in range(H)]
            Et1 = [[None] * NT for _ in range(H)]
            Et2 = [[None] * NT for _ in range(H)]

            def qk_proj_chunk(h, part, ch):
                if part == 0 and ch == 0:
                    qt[h] = qkp.tile([128, N], fp8, tag="qt", bufs=3, name=f"qt_{h}")
                    kt[h] = qkp.tile([128, N], fp8, tag="kt", bufs=3, name=f"kt_{h}")
                    dcol[h] = small.tile([128, 2 * NT], f32, tag="dcol", bufs=3, name=f"d_{h}")
                    rcol[h] = small.tile([128, 2 * NT], f32, tag="rcol", bufs=3, name=f"r_{h}")
                dst, w_dr, pb = (qt[h], wq_dr, 0) if part == 0 else (kt[h], wk_dr, 8)
                pq = psMM()
                for g in range(2):
                    nc.tensor.matmul(
                        pq[:],
                        w_dr[g][:, :, h * 128:(h + 1) * 128],
                        lnxT[:, 2 * g:2 * g + 2, ch * 512:(ch + 1) * 512],
                        start=(g == 0), stop=(g == 1), perf_mode=PM.DoubleRow)
                if ch == 0:
                    nc.vector.tensor_scalar(dst[:, ch * 512:(ch + 1) * 512], pq[:],
                                            CQ, bqpS[:, pb + h:pb + h + 1],
                                            ALU.mult, ALU.add)
                else:
                    nc.scalar.activation(dst[:, ch * 512:(ch + 1) * 512], pq[:],
                                         AF.Identity,
                                         bias=bqpS[:, pb + h:pb + h + 1], scale=CQ)

            def qk_proj_head(h, part):
                qk_proj_chunk(h, part, 0)
                qk_proj_chunk(h, part, 1)

            def v_proj(h, j):
                # computes m-tiles 2j, 2j+1 and fills Vp[h][j] in one drain op
                if j == 0:
                    Vp[h] = [vp.tile([128, 2, 512], fp8, tag="vp", bufs=12,
                                     name=f"v_{h}_{jj}") for jj in range(4)]
                for i in range(2):
                    mt = 2 * j + i
                    pv = psMM()
                    for g in range(2):
                        nc.tensor.matmul(pv[:],
                                         lnxT[:, 2 * g:2 * g + 2, mt * 128:(mt + 1) * 128],
                                         wv_dr[h][g][:, :, :],
                                         start=(g == 0), stop=(g == 1), perf_mode=PM.DoubleRow)
                    nc.vector.tensor_scalar(Vp[h][j][:, i, :], pv[:], CV, None, ALU.mult)

            pend = {}

            def scores_mm1(h, nt):
                s1 = psS()
                for c in range(2):
                    nc.tensor.matmul(s1[:, c * 512:(c + 1) * 512],
                                     qt[h][0:64, nt * 128:(nt + 1) * 128],
                                     kt[h][0:64, c * 512:(c + 1) * 512],
                                     start=True, stop=True)
                pend[(h, nt)] = s1

            def scores_mm2(h, nt):
                s2 = psS()
                for c in range(2):
                    nc.tensor.matmul(s2[:, c * 512:(c + 1) * 512],
                                     qt[h][64:128, nt * 128:(nt + 1) * 128],
                                     kt[h][64:128, c * 512:(c + 1) * 512],
                                     start=True, stop=True)
                pend[(h, nt)] = (pend[(h, nt)], s2)

            def exp_u(h, nt):
                s1, s2 = pend.pop((h, nt))
                E1 = ep.tile([128, N], fp8, tag="e1", bufs=12, name="E1")
                nc.scalar.activation(E1[:], s1[:], AF.Exp, scale=ESC,
                                     accum_out=dcol[h][:, 2 * nt:2 * nt + 1])
                E2 = ep.tile([128, N], fp8, tag="e2", bufs=12, name="E2")
                nc.scalar.activation(E2[:], s2[:], AF.Exp, scale=ESC,
                                     accum_out=dcol[h][:, 2 * nt + 1:2 * nt + 2])
                nc.vector.reciprocal(rcol[h][:, 2 * nt:2 * nt + 2],
                                     dcol[h][:, 2 * nt:2 * nt + 2])
                Et1[h][nt], Et2[h][nt] = E1, E2
                dg = dgp.tile([128, 128], bf16, tag="dg", bufs=16, name=f"dg_{h}_{nt}")
                nc.gpsimd.tensor_scalar(dg[:], ident_bf[:],
                                        rcol[h][:, 2 * nt:2 * nt + 1], SU,
                                        ALU.mult, ALU.mult)
                diag[h][nt] = dg
                dg2 = dgp.tile([128, 128], bf16, tag="dg", bufs=16, name=f"dg2_{h}_{nt}")
                nc.gpsimd.tensor_scalar(dg2[:], ident_bf[:],
                                        rcol[h][:, 2 * nt + 1:2 * nt + 2], -lam * SU,
                                        ALU.mult, ALU.mult)
                diag2[h][nt] = dg2

            def ut_transpose(h, k, half):
                # transposes U^T for m-tiles 2k, 2k+1, one [128,1024] drain op
                if k == 0 and half == 0:
                    UTp[h] = [utp.tile([128, 2, N], fp8, tag="utt", bufs=8,
                                       name=f"UT_{h}_{j}") for j in range(4)]
                for i in range(2):
                    mt = 2 * k + i
                    pu = psUT()
                    for q in range(4):
                        nt = half * 4 + q
                        # out[k, j] = (E1[j,k]*SU/d1[j]) - lam*(E2[j,k]*SU/d2[j])
                        nc.tensor.matmul(pu[:, q * 128:(q + 1) * 128],
                                         Et1[h][nt][:, mt * 128:(mt + 1) * 128],
                                         diag[h][nt][:], start=True, stop=False)
                        nc.tensor.matmul(pu[:, q * 128:(q + 1) * 128],
                                         Et2[h][nt][:, mt * 128:(mt + 1) * 128],
                                         diag2[h][nt][:], start=False, stop=True)
                    nc.vector.tensor_copy(UTp[h][k][:, i, half * 512:(half + 1) * 512],
                                          pu[:])

            def attnv(h, chunk):
                ct, chn = chunk % 4, chunk // 4
                if True:
                    po = psMM()
                    for j in range(4):
                        nc.tensor.matmul(po[:],
                                         Vp[h][j][:, :, ct * 128:(ct + 1) * 128],
                                         UTp[h][j][:, :, chn * 512:(chn + 1) * 512],
                                         start=(j == 0), stop=(j == 3), perf_mode=PM.DoubleRow)
                    nc.vector.tensor_scalar(OTp[h][ct // 2][:, ct % 2, chn * 512:(chn + 1) * 512],
                                            po[:], CO, bvvS[:, h * 4 + ct:h * 4 + ct + 1],
                                            ALU.mult, ALU.add)

            qk_proj_head(0, 0)
            qk_proj_head(0, 1)
            qk_proj_head(1, 0)
            qk_proj_head(1, 1)

            hT = lnxT  # reuse: lnxT is dead after the last projections
            x2p = [None] * NT

            lnf_rs = small.tile([128, 2 * NT], f32, name="lnf_rs")

            tailps = {}

            def tail_tile(key, tag):
                # one persistent [128,N] tile on a dead score tag; callers
                # double-buffer via 512-wide slices (slice-level deps)
                if key not in tailps:
                    tailps[key] = ps.tile([128, N], f32, tag=tag, bufs=1, name=key)
                return tailps[key]

            def merge_p1(nt):
                pm = psMM()[:]
                for hh in range(H):
                    for j in range(2):
                        nc.tensor.matmul(pm, OTp[hh][j][:, :, nt * 128:(nt + 1) * 128],
                                         wm_dr[hh][j][:, :, :],
                                         start=(hh == 0 and j == 0),
                                         stop=(hh == 7 and j == 1),
                                         perf_mode=PM.DoubleRow)
                xq = up.tile([128, D], f32, tag="u", bufs=8, name=f"x2p_{nt}")
                nc.vector.scalar_tensor_tensor(xq[:], pm, CM, xtm[nt],
                                               ALU.mult, ALU.add)
                x2p[nt] = xq
                st6 = small.tile([128, 6], f32, tag="st6")
                nc.vector.bn_stats(out=st6[:], in_=xq[:])
                mv = small.tile([128, 2], f32, tag="mv")
                nc.vector.bn_aggr(out=mv[:], in_=st6[:])
                rs = lnf_rs[:, 2 * nt:2 * nt + 1]
                rstd_ln(rs, mv[:, 1:2])
                nc.vector.tensor_scalar(lnf_rs[:, 2 * nt + 1:2 * nt + 2], mv[:, 0:1],
                                        rs, -1.0, ALU.mult, ALU.mult)

            def merge_p2(nt):
                xn = small.tile([128, D], bf16, tag="xnorm", bufs=2)
                nc.vector.tensor_scalar(xn[:], x2p[nt][:], lnf_rs[:, 2 * nt:2 * nt + 1],
                                        lnf_rs[:, 2 * nt + 1:2 * nt + 2], ALU.mult, ALU.add)
                trp = psTR()
                for ft in range(FT):
                    nc.tensor.transpose(trp[:, ft, :], xn[:, ft * 128:(ft + 1) * 128],
                                        ident_bf[:])
                dest = hT[:, :, nt * 128:(nt + 1) * 128]
                nc.vector.tensor_copy(dest, trp[:])

            for h in range(H + 1):
                for nt in range(NT):
                    if h == 0 and nt == 0:
                        scores_mm1(0, 0)
                        scores_mm2(0, 0)
                    nh, nn = (h, nt + 1) if nt < 7 else (h + 1, 0)
                    if h < H and nh < H:
                        scores_mm1(nh, nn)
                    if h < H:
                        exp_u(h, nt)
                    if h == 0 and nt % 2 == 1:
                        v_proj(0, nt // 2)
                    if h + 1 < H and nt % 2 == 1:
                        v_proj(h + 1, nt // 2)
                    if h < H and nt >= 4:
                        ut_transpose(h, nt - 4, 0)
                    if h >= 1 and nt < 4:
                        ut_transpose(h - 1, nt, 1)
                    if h < H and nh < H:
                        scores_mm2(nh, nn)
                    if h >= 1 and h < H:
                        attnv(h - 1, nt)
                    if h == H:
                        # head 7: chn0 chunks at nt<4; chn1 chunks pulled to
                        # nt 4-5 (UT half-1 ready after nt 3) so merge_p1 of
                        # tiles 4..7 can run in-loop at nt 6-7.
                        if nt < 4:
                            attnv(7, nt)
                        elif nt == 4:
                            attnv(7, 4)
                            attnv(7, 5)
                        elif nt == 5:
                            attnv(7, 6)
                            attnv(7, 7)
                    if h == H and nt >= 4:
                        merge_p1(nt - 4)
                    if h == H and nt >= 6:
                        merge_p1(nt - 2)
                        merge_p1(nt)
                    if h == H and nt >= 5:
                        merge_p2(nt - 5)
                    # deferred weight DMAs / qk projections
                    if nt == 1 and h + 2 < H:
                        qk_proj_head(h + 2, 0)
                    if nt == 5 and h + 2 < H:
                        qk_proj_head(h + 2, 1)
                    # xtm becomes x + (tp + bt2 + bm) broadcast, in place
                    if h == 6 and nt < 4:
                        nc.gpsimd.tensor_add(xtm[nt], xtm[nt], TP1[:])
                    if h == 7 and nt >= 4:
                        nc.gpsimd.tensor_add(xtm[nt], xtm[nt], TP1[:])

            merge_p2(3)

            # ---------------- FFN (silu table) -------------------------------
            at_pair = [utp.tile([128, 2, N], fp8, tag="utt", bufs=8, name=f"at_{j}")
                       for j in range(8)]

            def ffn1_op(ch, dblk):
                pf = psMM()[:]
                for g in range(2):
                    nc.tensor.matmul(pf, wf1_dr[g][:, :, dblk * 128:(dblk + 1) * 128],
                                     hT[:, 2 * g:2 * g + 2, ch * 512:(ch + 1) * 512],
                                     start=(g == 0), stop=(g == 1),
                                     perf_mode=PM.DoubleRow)
                sg = small.tile([128, 512], bf16, tag="sg", bufs=2)
                nc.scalar.activation(sg[:], pf, AF.Silu, scale=CF1,
                                     bias=bf1_c[:, dblk:dblk + 1])
                eng = nc.vector if dblk % 2 == 0 else nc.gpsimd
                eng.tensor_scalar(at_pair[dblk // 2][:, dblk % 2, ch * 512:(ch + 1) * 512],
                                  sg[:], SA, None, ALU.mult)

            def ffn2_op(nt):
                py_ = psMM()[:]
                for j in range(8):
                    nc.tensor.matmul(py_, at_pair[j][:, :, nt * 128:(nt + 1) * 128],
                                     wf2_dr[j][:, :, :],
                                     start=(j == 0), stop=(j == 7), perf_mode=PM.DoubleRow)
                yt = small.tile([128, D], f32, tag="yt", bufs=2)
                nc.vector.scalar_tensor_tensor(yt[:], py_, CF2, TPy[:], ALU.mult, ALU.add)
                nc.vector.tensor_add(yt[:], yt[:], x2p[nt][:])
                dma(y_d[nt * 128:(nt + 1) * 128, :], yt[:])

            for k in range(4):
                merge_p2(4 + k)
                for dblk in range(4 * k, 4 * k + 4):
                    ffn1_op(0, dblk)
            for k in range(4):
                for dblk in range(4 * k, 4 * k + 4):
                    ffn1_op(1, dblk)
                ffn2_op(k)
            for nt in range(4, NT):
                ffn2_op(nt)


_NC_CACHE = {}


def _get_nc(lam: float):
    key = float(lam)
    if key not in _NC_CACHE:
        _NC_CACHE[key] = build_program(key)
    return _NC_CACHE[key]


def kernel(**inputs) -> np.ndarray:
    from concourse.bass_utils import run_bass_kernel_spmd

    lam, in_maps = prepare_inputs(inputs)
    nc = _get_nc(lam)
    res = run_bass_kernel_spmd(nc, in_maps, core_ids=list(range(B)))
    return np.stack([res.results[b]["y"] for b in range(B)], axis=0).astype(np.float32)


# revision 43
# speedup vs baseline: 1.0178x; 1.0178x over previous
"""Trainium2 Bass kernel for nn_AttentionWithTime (differential-attention block).

Sharding: data-parallel over batch B=8 -> one batch element per NeuronCore,
no collectives.

v3: all weights are quantized to fp8(e4m3), scaled, and packed into their
on-chip DoubleRow layouts on the HOST (numpy); LN1/LNf gain+bias are folded
into Wqkv/Wf1 host-side, and bias vectors arrive pre-transposed into
per-partition columns.  The device kernel just DMAs weights straight into
their resident SBUF tiles (7.3MB fp8 instead of 29MB f32) and spends no
engine time on weight casts or bias transposes.

Attention pipeline (per head h, query tile nt):
  scores/exp/U (h, nt) | v-proj (h+1, nt) | qk-proj (h+2) | U^T built with PE
  transposes grouped per key-tile (each PSUM->SBUF copy moves [128,512])
  | attn@v (h-1) -> merge (PSUM-accumulated over heads) + LNf -> FFN.
fp8 tensors carry power-of-2 scales folded into the host-packed weights.
"""
import numpy as np

import concourse.bass as bass
import concourse.mybir as mybir
import concourse.tile as tile
from concourse import bacc
from concourse.masks import make_identity

B, N, D, H, DH, DE, DT = 8, 1024, 512, 8, 64, 2048, 256
DQKV = 6144
NT = N // 128
FT = D // 128
EPS = 1e-5
SCALE = DH ** -0.5

SX = 32.0
SQK = 32.0
SU = 4096.0
SV = 64.0
SO = 512.0
SA = 32.0
SW_QKV = 4096.0
SW_M = 8192.0
SW_F1 = 4096.0
SW_F2 = 8192.0

CQ = SQK / (SX * SW_QKV)
ESC = SCALE / (SQK * SQK)
CV = SV / (SX * SW_QKV)
CO = SO / (SU * SV)
CM = 1.0 / (SO * SW_M)
CF1 = 1.0 / (SX * SW_F1)
CF2 = 1.0 / (SA * SW_F2)

f32 = mybir.dt.float32
bf16 = mybir.dt.bfloat16
fp8 = mybir.dt.float8e4
AF = mybir.ActivationFunctionType
ALU = mybir.AluOpType
PM = mybir.MatmulPerfMode

FP8NP = mybir.dt.np(fp8)


# ---------------------------------------------------------------------------
# host-side input prep: fold LN affine params into weights, quantize to fp8,
# pack into the exact SBUF tile layouts, pre-transpose bias vectors.
# ---------------------------------------------------------------------------
def prepare_inputs(inputs):
    f = lambda k: np.asarray(inputs[k], np.float32)
    lam = float(np.asarray(inputs["lam"]))
    Wqkv, bqkv = f("Wqkv"), f("bqkv")
    g1, b1 = f("ln1_g"), f("ln1_b")
    Wq_eff = g1[:, None] * Wqkv
    bq_eff = bqkv + b1 @ Wqkv
    Wm = f("Wm")
    Wf1, bf1 = f("Wf1"), f("bf1")
    gf, bfb = f("lnf_g"), f("lnf_b")
    Wf1_eff = gf[:, None] * Wf1
    bf1_eff = bf1 + bfb @ Wf1
    Wf2 = f("Wf2")

    # qk columns packed per head: dcol = hh*128 + sub*64 <- scol = sub*512 + hh*64
    wqp = np.empty((2, 128, 2, 1024), np.float32)
    wkp = np.empty((2, 128, 2, 1024), np.float32)
    for g in range(2):
        for i in range(2):
            rows = Wq_eff[(2 * g + i) * 128:(2 * g + i + 1) * 128]
            for hh in range(8):
                for sub in range(2):
                    dcol = hh * 128 + sub * 64
                    scol = sub * 512 + hh * 64
                    wqp[g, :, i, dcol:dcol + 64] = rows[:, scol:scol + 64]
                    wkp[g, :, i, dcol:dcol + 64] = rows[:, 1024 + scol:1024 + scol + 64]
    wqp = (wqp * SW_QKV).astype(FP8NP)
    wkp = (wkp * SW_QKV).astype(FP8NP)

    wvp = np.empty((H, 2, 128, 2, 512), np.float32)
    for h in range(H):
        for g in range(2):
            for i in range(2):
                wvp[h, g, :, i, :] = Wq_eff[(2 * g + i) * 128:(2 * g + i + 1) * 128,
                                            2048 + h * 512:2048 + (h + 1) * 512]
    wvp = (wvp * SW_QKV).astype(FP8NP)

    wmp = np.empty((H, 2, 128, 2, 512), np.float32)
    for h in range(H):
        for j in range(2):
            for i in range(2):
                r = (4 * h + 2 * j + i) * 128
                wmp[h, j, :, i, :] = Wm[r:r + 128, :]
    wmp = (wmp * SW_M).astype(FP8NP)

    wf1p = np.empty((2, 128, 2, DE), np.float32)
    for g in range(2):
        for i in range(2):
            wf1p[g, :, i, :] = Wf1_eff[(2 * g + i) * 128:(2 * g + i + 1) * 128]
    wf1p = (wf1p * SW_F1).astype(FP8NP)

    wf2p = np.empty((8, 128, 2, 512), np.float32)
    for j in range(8):
        for i in range(2):
            wf2p[j, :, i, :] = Wf2[(2 * j + i) * 128:(2 * j + i + 1) * 128]
    wf2p = (wf2p * SW_F2).astype(FP8NP)

    # bias columns (pre-transposed to per-partition layout)
    bqps = np.empty((128, 16), np.float32)
    for p in range(16):
        hh, base = p % 8, (0 if p < 8 else 1024)
        bqps[0:64, p] = bq_eff[base + hh * 64:base + (hh + 1) * 64]
        bqps[64:128, p] = bq_eff[base + 512 + hh * 64:base + 512 + (hh + 1) * 64]
    bqps *= SQK
    bvvs = bq_eff[2048:6144].reshape(32, 128).T.copy() * ((1.0 - lam) * SO)
    bf1c = bf1_eff.reshape(16, 128).T.copy()
    bt1c = f("bt1").reshape(2, 128).T.copy()

    # partition-major single-DMA blobs
    wqh = np.ascontiguousarray(wqp.transpose(1, 0, 2, 3).reshape(128, 2, 2048))
    wkh = np.ascontiguousarray(wkp.transpose(1, 0, 2, 3).reshape(128, 2, 2048))
    wvh = np.ascontiguousarray(
        wvp.transpose(2, 0, 1, 3, 4).reshape(128, 16, 1024))
    wmh = np.ascontiguousarray(
        wmp.transpose(2, 0, 1, 3, 4).reshape(128, 16, 1024))
    wf1h = np.ascontiguousarray(wf1p.transpose(1, 0, 2, 3).reshape(128, 2, 2 * DE))
    wf2h = np.ascontiguousarray(wf2p.transpose(1, 0, 2, 3).reshape(128, 8, 1024))
    cvec = np.concatenate([bqps, bvvs, bf1c, bt1c], axis=1).astype(np.float32)  # [128, 66]
    rows3 = np.ascontiguousarray(
        np.concatenate([f("bm"), f("bt2"), f("bf2")])[None, :])  # [1, 1536]
    wt1h = np.ascontiguousarray(f("Wt1").reshape(2, 128, DT).transpose(1, 0, 2))
    wt2h = np.ascontiguousarray(f("Wt2").reshape(2, 128, D).transpose(1, 0, 2))
    shared = {
        "wqh": wqh, "wkh": wkh, "wvh": wvh, "wmh": wmh, "wf1h": wf1h,
        "wf2h": wf2h, "cvec": cvec, "rows3": rows3,
        "wt1h": wt1h, "wt2h": wt2h,
    }
    x = f("x")
    t = f("t")
    in_maps = []
    for b in range(B):
        m = dict(shared)
        m["x"] = np.ascontiguousarray(x[b])
        m["cvec"] = np.ascontiguousarray(np.concatenate(
            [cvec, t[b].reshape(2, 128).T], axis=1))  # [128, 68]
        in_maps.append(m)
    return lam, in_maps


def build_program(lam: float):
    nc = bacc.Bacc("TRN2", target_bir_lowering=False, debug=False, num_devices=8)

    d = {}
    d["x_d"] = nc.dram_tensor("x", [N, D], f32, kind="ExternalInput")
    d["wqh_d"] = nc.dram_tensor("wqh", [128, 2, 2048], fp8, kind="ExternalInput")
    d["wkh_d"] = nc.dram_tensor("wkh", [128, 2, 2048], fp8, kind="ExternalInput")
    d["wvh_d"] = nc.dram_tensor("wvh", [128, 16, 1024], fp8, kind="ExternalInput")
    d["wmh_d"] = nc.dram_tensor("wmh", [128, 16, 1024], fp8, kind="ExternalInput")
    d["wf1h_d"] = nc.dram_tensor("wf1h", [128, 2, 2 * DE], fp8, kind="ExternalInput")
    d["wf2h_d"] = nc.dram_tensor("wf2h", [128, 8, 1024], fp8, kind="ExternalInput")
    d["cvec_d"] = nc.dram_tensor("cvec", [128, 68], f32, kind="ExternalInput")
    d["rows3_d"] = nc.dram_tensor("rows3", [1, 1536], f32, kind="ExternalInput")
    d["wt1h_d"] = nc.dram_tensor("wt1h", [128, 2, DT], f32, kind="ExternalInput")
    d["wt2h_d"] = nc.dram_tensor("wt2h", [128, 2, D], f32, kind="ExternalInput")
    d["y_d"] = nc.dram_tensor("y", [N, D], f32, kind="ExternalOutput")

    with tile.TileContext(nc) as tc:
        _build(tc, lam, d)
    nc.compile()
    return nc


def _build(tc, lam, d):
    nc = tc.nc
    dma = nc.sync.dma_start

    from contextlib import ExitStack
    with ExitStack() as es:
        cst = es.enter_context(tc.tile_pool(name="cst", bufs=1))
        small = es.enter_context(tc.tile_pool(name="small", bufs=8))
        otp = es.enter_context(tc.tile_pool(name="otp", bufs=1))
        ps = es.enter_context(tc.tile_pool(name="ps", bufs=1, space="PSUM"))

        _scnt = [0]

        def psS():
            _scnt[0] += 1
            return ps.tile([128, N], f32, tag=f"s{_scnt[0] % 3}", bufs=1, name="psS")

        def psUT():
            return ps.tile([128, 512], f32, tag="mm", bufs=2, name="psUT")

        def psTR():
            return ps.tile([128, FT, 128], bf16, tag="mm", bufs=2, name="psTR")

        def psMM():
            return ps.tile([128, 512], f32, tag="mm", bufs=2, name="psMM")

        def rstd_ln(dst, var_ap):
            # dst = SX / sqrt(var + eps):  sqrt((var+eps)/SX^2) then recip
            nc.scalar.activation(dst, var_ap, AF.Sqrt, bias=eps_c[:],
                                 scale=1.0 / (SX * SX))
            nc.vector.reciprocal(dst, dst)

        # ---------------- constants ----------------
        ident_bf = cst.tile([128, 128], bf16)
        make_identity(nc, ident_bf[:])
        ones1 = cst.tile([1, 128], f32)
        nc.gpsimd.memset(ones1[:], 1.0)
        eps_c = cst.tile([128, 1], f32)
        nc.gpsimd.memset(eps_c[:], EPS / (SX * SX))

        # ---------------- input/weight DMAs: one blob per tensor --------------
        x_d, y_d = d["x_d"], d["y_d"]

        xall = cst.tile([128, NT, D], f32, name="xall")
        xv = x_d[:].rearrange("(a p) c -> p a c", p=128)
        for nt in range(NT):
            dma(xall[:, nt, :], xv[:, nt, :])
        xtm = [xall[:, nt, :] for nt in range(NT)]

        cv = cst.tile([128, 68], f32, name="cvec")
        dma(cv[:], d["cvec_d"][:])
        bqpS = cv[:, 0:16]
        bvvS = cv[:, 16:48]
        bf1_c = cv[:, 48:64]
        bt1_c = cv[:, 64:66]
        tT = cv[:, 66:68]

        wq_all = cst.tile([128, 2, 2048], fp8, name="wq")
        dma(wq_all[:], d["wqh_d"][:])
        wk_all = cst.tile([128, 2, 2048], fp8, name="wk")
        dma(wk_all[:], d["wkh_d"][:])
        wv_all = cst.tile([128, 16, 1024], fp8, name="wv")
        dma(wv_all[:], d["wvh_d"][:])
        wm_all = cst.tile([128, 16, 1024], fp8, name="wm")
        dma(wm_all[:], d["wmh_d"][:])
        wf1_all = cst.tile([128, 2, 2 * DE], fp8, name="wf1")
        dma(wf1_all[:], d["wf1h_d"][:])
        wf2_all = cst.tile([128, 8, 1024], fp8, name="wf2")
        dma(wf2_all[:], d["wf2h_d"][:])

        r2 = lambda ap: ap.rearrange("p (i c) -> p i c", i=2)
        wq_dr = [r2(wq_all[:, g, :]) for g in range(2)]
        wk_dr = [r2(wk_all[:, g, :]) for g in range(2)]
        wv_dr = [[r2(wv_all[:, 2 * h + g, :]) for g in range(2)] for h in range(H)]
        wm_dr = [[r2(wm_all[:, 2 * h + j, :]) for j in range(2)] for h in range(H)]
        wf1_dr = [r2(wf1_all[:, g, :]) for g in range(2)]
        wf2_dr = [r2(wf2_all[:, j, :]) for j in range(8)]

        # ---------------- LN1 (exp table: rstd via ln+exp) -------------------
        lnxT = cst.tile([128, FT, N], fp8, name="lnxT")

        def layer_norm_tile(src, dest_all, nt):
            st6 = small.tile([128, 6], f32, tag="st6")
            nc.vector.bn_stats(out=st6[:], in_=src)
            mv = small.tile([128, 2], f32, tag="mv")
            nc.vector.bn_aggr(out=mv[:], in_=st6[:])
            rstd = small.tile([128, 1], f32, tag="rstd")
            rstd_ln(rstd[:], mv[:, 1:2])
            nm = small.tile([128, 1], f32, tag="nm")
            nc.vector.tensor_scalar(nm[:], mv[:, 0:1], rstd[:], -1.0, ALU.mult, ALU.mult)
            xn = small.tile([128, D], bf16, tag="xnorm", bufs=2)
            nc.vector.tensor_scalar(xn[:], src, rstd[:], nm[:], ALU.mult, ALU.add)
            trp = psTR()
            for ft in range(FT):
                nc.tensor.transpose(trp[:, ft, :], xn[:, ft * 128:(ft + 1) * 128],
                                    ident_bf[:])
            dest = dest_all[:, :, nt * 128:(nt + 1) * 128]
            if nt % 2 == 0:
                nc.vector.tensor_copy(dest, trp[:])
            else:
                nc.scalar.activation(dest, trp[:], AF.Identity)

        for nt in range(NT):
            layer_norm_tile(xtm[nt], lnxT, nt)

        # ---------------- time MLP ------------------------------------------
        TP1 = cst.tile([128, D], f32)
        TPy = cst.tile([128, D], f32)
        with tc.tile_pool(name="trow", bufs=1) as trow:
            wt1 = trow.tile([128, 2, DT], f32, name="wt1")
            nc.scalar.dma_start(wt1[:], d["wt1h_d"][:])
            wt2 = trow.tile([128, 2, D], f32, name="wt2")
            nc.scalar.dma_start(wt2[:], d["wt2h_d"][:])
            rows3 = trow.tile([1, 3 * D], f32, name="rows3")
            nc.scalar.dma_start(rows3[:], d["rows3_d"][:])
            bm_r = rows3[0:1, 0:D]
            bt2_r = rows3[0:1, D:2 * D]
            bf2_r = rows3[0:1, 2 * D:3 * D]
            s_cols = []
            for dc in range(2):
                l1_ps = psMM()
                for ft in range(2):
                    nc.tensor.matmul(l1_ps[:, 0:1],
                                     wt1[:, ft, dc * 128:(dc + 1) * 128],
                                     tT[:, ft:ft + 1], start=(ft == 0), stop=(ft == 1))
                s_c = small.tile([128, 1], f32, tag="s_col")
                nc.scalar.activation(s_c[:], l1_ps[:, 0:1], AF.Silu, bias=bt1_c[:, dc:dc + 1])
                s_cols.append(s_c)
            tp_ps = psMM()
            for dc in range(2):
                nc.tensor.matmul(tp_ps[0:1, :], s_cols[dc][:], wt2[:, dc, :],
                                 start=(dc == 0), stop=(dc == 1))
            row1 = trow.tile([1, D], f32)
            nc.vector.tensor_add(row1[:], tp_ps[0:1, :], bt2_r)
            rowy = trow.tile([1, D], f32)
            nc.vector.tensor_sub(rowy[:], bf2_r, row1[:])
            nc.vector.tensor_add(row1[:], row1[:], bm_r)
            for row, TP in ((row1, TP1), (rowy, TPy)):
                tp_b = psMM()
                nc.tensor.matmul(tp_b[:], ones1[:], row[:], start=True, stop=True)
                nc.vector.tensor_copy(TP[:], tp_b[:])

        # ---------------- attention phase -----------------------------------
        OTp = [[otp.tile([128, 2, N], fp8, tag="ot", bufs=16, name=f"OT_{h}_{j}")
                for j in range(2)] for h in range(H)]

        with ExitStack() as esA:
            qkp = esA.enter_context(tc.tile_pool(name="qkp", bufs=1))
            ep = esA.enter_context(tc.tile_pool(name="ep", bufs=1))
            up = esA.enter_context(tc.tile_pool(name="up", bufs=1))
            utp = esA.enter_context(tc.tile_pool(name="utp", bufs=1))
            vp = esA.enter_context(tc.tile_pool(name="vp", bufs=1))
            dgp = esA.enter_context(tc.tile_pool(name="dgp", bufs=1))

            qt = [None] * H
            kt = [None] * H
            UTp = [None] * H
            Vp = [None] * H
            dcol = [None] * H
            rcol = [None] * H
            diag = [[None] * NT for _ in range(H)]
            diag2 = [[None] * NT for _ in range(H)]
            Et1 = [[None] * NT for _ in range(H)]
            Et2 = [[None] * NT for _ in range(H)]

            def qk_proj_chunk(h, part, ch):
                if part == 0 and ch == 0:
                    qt[h] = qkp.tile([128, N], fp8, tag="qt", bufs=3, name=f"qt_{h}")
                    kt[h] = qkp.tile([128, N], fp8, tag="kt", bufs=3, name=f"kt_{h}")
                    dcol[h] = small.tile([128, 2 * NT], f32, tag="dcol", bufs=3, name=f"d_{h}")
                    rcol[h] = small.tile([128, 2 * NT], f32, tag="rcol", bufs=3, name=f"r_{h}")
                dst, w_dr, pb = (qt[h], wq_dr, 0) if part == 0 else (kt[h], wk_dr, 8)
                pq = psMM()
                for g in range(2):
                    nc.tensor.matmul(
                        pq[:],
                        w_dr[g][:, :, h * 128:(h + 1) * 128],
                        lnxT[:, 2 * g:2 * g + 2, ch * 512:(ch + 1) * 512],
                        start=(g == 0), stop=(g == 1), perf_mode=PM.DoubleRow)
                nc.vector.tensor_scalar(dst[:, ch * 512:(ch + 1) * 512], pq[:],
                                        CQ, bqpS[:, pb + h:pb + h + 1],
                                        ALU.mult, ALU.add)

            def qk_proj_head(h, part):
                qk_proj_chunk(h, part, 0)
                qk_proj_chunk(h, part, 1)

            def v_proj(h, j):
                # computes m-tiles 2j, 2j+1 and fills Vp[h][j] in one drain op
                if j == 0:
                    Vp[h] = [vp.tile([128, 2, 512], fp8, tag="vp", bufs=12,
                                     name=f"v_{h}_{jj}") for jj in range(4)]
                for i in range(2):
                    mt = 2 * j + i
                    pv = psMM()
                    for g in range(2):
                        nc.tensor.matmul(pv[:],
                                         lnxT[:, 2 * g:2 * g + 2, mt * 128:(mt + 1) * 128],
                                         wv_dr[h][g][:, :, :],
                                         start=(g == 0), stop=(g == 1), perf_mode=PM.DoubleRow)
                    nc.vector.tensor_scalar(Vp[h][j][:, i, :], pv[:], CV, None, ALU.mult)

            pend = {}

            def scores_mm1(h, nt):
                s1 = psS()
                for c in range(2):
                    nc.tensor.matmul(s1[:, c * 512:(c + 1) * 512],
                                     qt[h][0:64, nt * 128:(nt + 1) * 128],
                                     kt[h][0:64, c * 512:(c + 1) * 512],
                                     start=True, stop=True)
                pend[(h, nt)] = s1

            def scores_mm2(h, nt):
                s2 = psS()
                for c in range(2):
                    nc.tensor.matmul(s2[:, c * 512:(c + 1) * 512],
                                     qt[h][64:128, nt * 128:(nt + 1) * 128],
                                     kt[h][64:128, c * 512:(c + 1) * 512],
                                     start=True, stop=True)
                pend[(h, nt)] = (pend[(h, nt)], s2)

            def exp_u(h, nt):
                s1, s2 = pend.pop((h, nt))
                E1 = ep.tile([128, N], fp8, tag="e1", bufs=12, name="E1")
                nc.scalar.activation(E1[:], s1[:], AF.Exp, scale=ESC,
                                     accum_out=dcol[h][:, 2 * nt:2 * nt + 1])
                E2 = ep.tile([128, N], fp8, tag="e2", bufs=12, name="E2")
                nc.scalar.activation(E2[:], s2[:], AF.Exp, scale=ESC,
                                     accum_out=dcol[h][:, 2 * nt + 1:2 * nt + 2])
                nc.vector.reciprocal(rcol[h][:, 2 * nt:2 * nt + 2],
                                     dcol[h][:, 2 * nt:2 * nt + 2])
                Et1[h][nt], Et2[h][nt] = E1, E2
                dg = dgp.tile([128, 128], bf16, tag="dg", bufs=16, name=f"dg_{h}_{nt}")
                nc.gpsimd.tensor_scalar(dg[:], ident_bf[:],
                                        rcol[h][:, 2 * nt:2 * nt + 1], SU,
                                        ALU.mult, ALU.mult)
                diag[h][nt] = dg
                dg2 = dgp.tile([128, 128], bf16, tag="dg", bufs=16, name=f"dg2_{h}_{nt}")
                nc.gpsimd.tensor_scalar(dg2[:], ident_bf[:],
                                        rcol[h][:, 2 * nt + 1:2 * nt + 2], -lam * SU,
                                        ALU.mult, ALU.mult)
                diag2[h][nt] = dg2

            def ut_transpose(h, k, half):
                # transposes U^T for m-tiles 2k, 2k+1, one [128,1024] drain op
                if k == 0 and half == 0:
                    UTp[h] = [utp.tile([128, 2, N], fp8, tag="utt", bufs=8,
                                       name=f"UT_{h}_{j}") for j in range(4)]
                for i in range(2):
                    mt = 2 * k + i
                    pu = psUT()
                    for q in range(4):
                        nt = half * 4 + q
                        # out[k, j] = (E1[j,k]*SU/d1[j]) - lam*(E2[j,k]*SU/d2[j])
                        nc.tensor.matmul(pu[:, q * 128:(q + 1) * 128],
                                         Et1[h][nt][:, mt * 128:(mt + 1) * 128],
                                         diag[h][nt][:], start=True, stop=False)
                        nc.tensor.matmul(pu[:, q * 128:(q + 1) * 128],
                                         Et2[h][nt][:, mt * 128:(mt + 1) * 128],
                                         diag2[h][nt][:], start=False, stop=True)
                    nc.vector.tensor_copy(UTp[h][k][:, i, half * 512:(half + 1) * 512],
                                          pu[:])

            def attnv(h, chunk):
                ct, chn = chunk % 4, chunk // 4
                if True:
                    po = psMM()
                    for j in range(4):
                        nc.tensor.matmul(po[:],
                                         Vp[h][j][:, :, ct * 128:(ct + 1) * 128],
                                         UTp[h][j][:, :, chn * 512:(chn + 1) * 512],
                                         start=(j == 0), stop=(j == 3), perf_mode=PM.DoubleRow)
                    nc.vector.tensor_scalar(OTp[h][ct // 2][:, ct % 2, chn * 512:(chn + 1) * 512],
                                            po[:], CO, bvvS[:, h * 4 + ct:h * 4 + ct + 1],
                                            ALU.mult, ALU.add)

            qk_proj_head(0, 0)
            qk_proj_head(0, 1)
            qk_proj_head(1, 0)
            qk_proj_head(1, 1)

            hT = lnxT  # reuse: lnxT is dead after the last projections
            x2p = [None] * NT

            lnf_rs = small.tile([128, 2 * NT], f32, name="lnf_rs")

            tailps = {}

            def tail_tile(key, tag):
                # one persistent [128,N] tile on a dead score tag; callers
                # double-buffer via 512-wide slices (slice-level deps)
                if key not in tailps:
                    tailps[key] = ps.tile([128, N], f32, tag=tag, bufs=1, name=key)
                return tailps[key]

            def merge_p1(nt):
                pm = psMM()[:]
                for hh in range(H):
                    for j in range(2):
                        nc.tensor.matmul(pm, OTp[hh][j][:, :, nt * 128:(nt + 1) * 128],
                                         wm_dr[hh][j][:, :, :],
                                         start=(hh == 0 and j == 0),
                                         stop=(hh == 7 and j == 1),
                                         perf_mode=PM.DoubleRow)
                xq = up.tile([128, D], f32, tag="u", bufs=8, name=f"x2p_{nt}")
                nc.vector.scalar_tensor_tensor(xq[:], pm, CM, xtm[nt],
                                               ALU.mult, ALU.add)
                x2p[nt] = xq
                st6 = small.tile([128, 6], f32, tag="st6")
                nc.vector.bn_stats(out=st6[:], in_=xq[:])
                mv = small.tile([128, 2], f32, tag="mv")
                nc.vector.bn_aggr(out=mv[:], in_=st6[:])
                rs = lnf_rs[:, 2 * nt:2 * nt + 1]
                rstd_ln(rs, mv[:, 1:2])
                nc.vector.tensor_scalar(lnf_rs[:, 2 * nt + 1:2 * nt + 2], mv[:, 0:1],
                                        rs, -1.0, ALU.mult, ALU.mult)

            def merge_p2(nt):
                xn = small.tile([128, D], bf16, tag="xnorm", bufs=2)
                nc.vector.tensor_scalar(xn[:], x2p[nt][:], lnf_rs[:, 2 * nt:2 * nt + 1],
                                        lnf_rs[:, 2 * nt + 1:2 * nt + 2], ALU.mult, ALU.add)
                trp = psTR()
                for ft in range(FT):
                    nc.tensor.transpose(trp[:, ft, :], xn[:, ft * 128:(ft + 1) * 128],
                                        ident_bf[:])
                dest = hT[:, :, nt * 128:(nt + 1) * 128]
                if nt % 2 == 0:
                    nc.vector.tensor_copy(dest, trp[:])
                else:
                    nc.scalar.activation(dest, trp[:], AF.Identity)

            for h in range(H + 1):
                for nt in range(NT):
                    if h == 0 and nt == 0:
                        scores_mm1(0, 0)
                        scores_mm2(0, 0)
                    nh, nn = (h, nt + 1) if nt < 7 else (h + 1, 0)
                    if h < H and nh < H:
                        scores_mm1(nh, nn)
                    if h < H:
                        exp_u(h, nt)
                    if h == 0 and nt % 2 == 1:
                        v_proj(0, nt // 2)
                    if h + 1 < H and nt % 2 == 1:
                        v_proj(h + 1, nt // 2)
                    if h < H and nt >= 4:
                        ut_transpose(h, nt - 4, 0)
                    if h >= 1 and nt < 4:
                        ut_transpose(h - 1, nt, 1)
                    if h < H and nh < H:
                        scores_mm2(nh, nn)
                    if h >= 1 and h < H:
                        attnv(h - 1, nt)
                    if h == H:
                        # head 7: chn0 chunks at nt<4; chn1 chunks pulled to
                        # nt 4-5 (UT half-1 ready after nt 3) so merge_p1 of
                        # tiles 4..7 can run in-loop at nt 6-7.
                        if nt < 4:
                            attnv(7, nt)
                        elif nt == 4:
                            attnv(7, 4)
                            attnv(7, 5)
                        elif nt == 5:
                            attnv(7, 6)
                            attnv(7, 7)
                    if h == H and nt >= 4:
                        merge_p1(nt - 4)
                    if h == H and nt >= 6:
                        merge_p1(nt - 2)
                        merge_p1(nt)
                    if h == H and nt >= 5:
                        merge_p2(nt - 5)
                    # deferred weight DMAs / qk projections
                    if nt == 1 and h + 2 < H:
                        qk_proj_head(h + 2, 0)
                    if nt == 5 and h + 2 < H:
                        qk_proj_head(h + 2, 1)
                    # xtm becomes x + (tp + bt2 + bm) broadcast, in place
                    if h == 6 and nt < 4:
                        nc.gpsimd.tensor_add(xtm[nt], xtm[nt], TP1[:])
                    if h == 7 and nt >= 4:
                        nc.gpsimd.tensor_add(xtm[nt], xtm[nt], TP1[:])

            merge_p2(3)

            # ---------------- FFN (silu table) -------------------------------
            at_pair = [utp.tile([128, 2, N], fp8, tag="utt", bufs=8, name=f"at_{j}")
                       for j in range(8)]

            def ffn1_op(ch, dblk):
                pf = psMM()[:]
                for g in range(2):
                    nc.tensor.matmul(pf, wf1_dr[g][:, :, dblk * 128:(dblk + 1) * 128],
                                     hT[:, 2 * g:2 * g + 2, ch * 512:(ch + 1) * 512],
                                     start=(g == 0), stop=(g == 1),
                                     perf_mode=PM.DoubleRow)
                sg = small.tile([128, 512], bf16, tag="sg", bufs=2)
                nc.scalar.activation(sg[:], pf, AF.Silu, scale=CF1,
                                     bias=bf1_c[:, dblk:dblk + 1])
                eng = nc.vector if dblk % 2 == 0 else nc.gpsimd
                eng.tensor_scalar(at_pair[dblk // 2][:, dblk % 2, ch * 512:(ch + 1) * 512],
                                  sg[:], SA, None, ALU.mult)

            def ffn2_op(nt):
                py_ = psMM()[:]
                for j in range(8):
                    nc.tensor.matmul(py_, at_pair[j][:, :, nt * 128:(nt + 1) * 128],
                                     wf2_dr[j][:, :, :],
                                     start=(j == 0), stop=(j == 7), perf_mode=PM.DoubleRow)
                yt = small.tile([128, D], f32, tag="yt", bufs=2)
                nc.vector.scalar_tensor_tensor(yt[:], py_, CF2, TPy[:], ALU.mult, ALU.add)
                nc.vector.tensor_add(yt[:], yt[:], x2p[nt][:])
                dma(y_d[nt * 128:(nt + 1) * 128, :], yt[:])

            for k in range(4):
                merge_p2(4 + k)
                for dblk in range(4 * k, 4 * k + 4):
                    ffn1_op(0, dblk)
            for k in range(4):
                for dblk in range(4 * k, 4 * k + 4):
                    ffn1_op(1, dblk)
                ffn2_op(k)
            for nt in range(4, NT):
                ffn2_op(nt)


_NC_CACHE = {}


def _get_nc(lam: float):
    key = float(lam)
    if key not in _NC_CACHE:
        _NC_CACHE[key] = build_program(key)
    return _NC_CACHE[key]


def kernel(**inputs) -> np.ndarray:
    from concourse.bass_utils import run_bass_kernel_spmd

    lam, in_maps = prepare_inputs(inputs)
    nc = _get_nc(lam)
    res = run_bass_kernel_spmd(nc, in_maps, core_ids=list(range(B)))
    return np.stack([res.results[b]["y"] for b in range(B)], axis=0).astype(np.float32)
